# revision 45
# baseline (speedup 1.0000x reference)
"""Dilated attention (banded local-window attention) for Trainium2.

Problem: q,k,v [1, 16, 4096, 64] fp32; dilation r=2, window 128 (band |i-j|<=64
within each of the 2 strided subsequences of length 2048 per head).

Sharding: 16 heads x 2 offsets = 32 independent blocks -> 8 cores x 2 heads
(4 blocks). The host-side shard step hands each core its q/k already in
d-major ("transposed") per-offset layout [head, r, d, i] so the device reads
Q^T/K^T with full-bandwidth contiguous descriptors; offset r=0 (block A)
lands on SBUF partitions 0:64 and r=1 (block B) on 64:128, which feeds
row-packed K=64 QK matmuls on the two halves of the PE array. All I/O is
bf16 on the wire (the kernel computes in bf16 anyway).

Per block, queries are tiled in 16 tiles of 128; each tile attends to a
256-key window (two 128-chunks at +-64 around the tile). Scores are computed
transposed (S^T[jj, i]) so the probabilities come out pre-transposed for the
PV matmul (no on-chip transpose of P). Softmax skips the max-subtraction
(scores ~ N(0,1) after the 1/8 scale, exp is safe) and folds the 1/8 scale
into the ScalarE exp. The band mask is a 0/1 bf16 multiply after exp. Row
sums come from a ones-column appended to V; out = (P@[V|1])[:,:64] *
1/(P@[V|1])[:,64]. The ones column is zero on the +-64-row halo around the
sequence, so out-of-range keys (zero-padded in kT, exp(0)=1 in P) contribute
nothing to either numerator or denominator -- one uniform band mask suffices
for every tile, no edge-tile variants.

Schedule: the 16 DMA hardware engines are shared by every DGE queue, so
input bytes form effectively ONE ~300GB/s stream; the only scheduling that
matters is (a) dispatch early, (b) order chunks by consumption. Inputs ride
the single SWDGE (gpsimd) ring (depth ~8 vs HWDGE's 2; each dispatch costs
~0.7us) as per-head chunks in consumption order -- q/k [0:384]/[384:1152],
v tiles [0:5]/[5:9] -- except the last-needed q/k chunks [1152:], which go
on the sync HWDGE queue emitted after g1 to relieve SWDGE dispatch pacing.
The two heads' group loops are interleaved so the PE always has the other
head's QK work while one head's exp->mask->PV chain drains. Scores PSUM is
triple-buffered and PV PSUM double-buffered (3*2 + 2*1 = 8 banks exactly).
The output is written in the SBUF-native [p, t, (r d)] layout (1KB
contiguous per partition per flush; the natural [S, D] layout would be a
128B-element scatter at ~1/8th DMA efficiency) and un-permuted on the
host; the final two groups flush 2 tiles each so almost nothing trails the
last matmul.

Dummy 1x1 matmuls ("absorbers") read one element of each freshly-loaded
chunk so the PE sequencer observes the DMA semaphores there; the real
matmuls then never combine a DMA wait with their steady-state PSUM-WAW
wait (MM ISA wait-slot limit is 2). Same trick as the DVE mdmy read for
the one-time Pool mask-init wait (TT limit is 1).
"""

import sys

for _p in ("/opt/trn_rl_repo", "/opt/trn_rl_repo/concourse"):
    if _p not in sys.path:
        sys.path.insert(0, _p)

import ml_dtypes
import numpy as np

import concourse.bass as bass
import concourse.mybir as mybir
import concourse.tile as tile
from concourse import bacc
from concourse.bass_utils import run_bass_kernel_spmd


def _ensure_axon_hooks():
    """run_bass_kernel_spmd's trace path hard-imports antenv.axon_hooks,
    which some agent images lack; with BASS_TRACE set that import crashes
    kernel(). Provide a compatible stand-in whose hook is None, which the
    trace path treats as 'profiling unavailable': tracing is skipped and
    the kernel still runs. Environments with the real module (or a shim
    installed by the caller beforehand) are untouched."""
    try:
        import antenv.axon_hooks  # noqa: F401
        return
    except ImportError:
        pass
    import types

    try:
        import antenv  # noqa: F401
    except ImportError:
        return
    hooks = types.ModuleType("antenv.axon_hooks")
    hooks._HOOK = None
    hooks.set_axon_ntff_profile_hook = lambda h: setattr(hooks, "_HOOK", h)
    hooks.get_axon_ntff_profile_hook = lambda: hooks._HOOK
    sys.modules["antenv.axon_hooks"] = hooks


_ensure_axon_hooks()

N_CORES = 8
B, H, S, D = 1, 16, 4096, 64
R = 2                      # dilation rate
NSEQ = S // R              # 2048 per-offset sequence length
HALF = 64                  # window//2
NT = NSEQ // 128           # 16 query tiles per block
HPC = H // N_CORES         # heads per core = 2

F32 = mybir.dt.float32
BF16 = mybir.dt.bfloat16

# q/k column chunks and v tile chunks, in consumption order. The first
# chunks (group 0: q cols <256, k cols <384, v tiles <5) ride the sync +
# scalar HWDGE queues, whose engines clear the framework preamble ~3us
# before gpsimd can issue its first SWDGE dispatch -- and the SWDGE
# stream hasn't started yet, so these bytes fly uncontended. Everything
# later streams on SWDGE in consumption order.
QK1, QK2, QK3 = 384, 640, 1152
V1, V2 = 5, 9


def _alloc_tiles(trans, vpool, opool):
    qTs = trans.tile([128, NSEQ], BF16, tag="qTs")
    kTp = trans.tile([128, NSEQ + 128], BF16, tag="kTp")
    vsh = vpool.tile([128, NT + 1, R, 65], BF16, tag="vsh")
    out_sb = opool.tile([128, NT, 128], BF16, tag="out_sb")
    return qTs, kTp, vsh, out_sb


def _emit_early_loads(nc, head_tiles, qT, kT, v, h):
    """Group-0 chunks on the HWDGE queues (3 dispatches each, within the
    depth-2 ring): sync carries h0's q+v and h1's k, scalar the mirror."""
    qTs, kTp, vsh, _ = head_tiles[h]
    qTr = qT[h].rearrange("r d i -> (r d) i")
    kTr = kT[h].rearrange("r d i -> (r d) i")
    eq, ek = (nc.sync, nc.scalar) if h == 0 else (nc.scalar, nc.sync)
    eq.dma_start(qTs[:, 0:256], qTr[:, 0:256])
    ek.dma_start(kTp[:, 0:QK1], kTr[:, 0:QK1])
    eq.dma_start(vsh[:, 0:V1], v[h, :, 0:V1])


def _emit_gp_stages(nc, head_tiles, qT, kT, v):
    """The rest of the input set, streamed on SWDGE in consumption order."""
    qk = []
    for h in range(HPC):
        qTs, kTp, vsh, _ = head_tiles[h]
        qk.append(
            (qTs, kTp, vsh,
             qT[h].rearrange("r d i -> (r d) i"),
             kT[h].rearrange("r d i -> (r d) i"))
        )
    for h in range(HPC):
        qTs, kTp, _, qTr, kTr = qk[h]
        nc.gpsimd.dma_start(qTs[:, 256:QK2], qTr[:, 256:QK2])
        nc.gpsimd.dma_start(kTp[:, QK1:QK2], kTr[:, QK1:QK2])
    for h in range(HPC):
        qTs, kTp, _, qTr, kTr = qk[h]
        nc.gpsimd.dma_start(qTs[:, QK2:QK3], qTr[:, QK2:QK3])
        nc.gpsimd.dma_start(kTp[:, QK2:QK3], kTr[:, QK2:QK3])
    for h in range(HPC):
        _, _, vsh, _, _ = qk[h]
        nc.gpsimd.dma_start(vsh[:, V1:V2], v[h, :, V1:V2])
    for h in range(HPC):
        qTs, kTp, vsh, qTr, kTr = qk[h]
        nc.gpsimd.dma_start(qTs[:, QK3:NSEQ], qTr[:, QK3:NSEQ])
        nc.gpsimd.dma_start(kTp[:, QK3 : NSEQ + 128], kTr[:, QK3 : NSEQ + 128])
        nc.gpsimd.dma_start(vsh[:, V2 : NT + 1], v[h, :, V2 : NT + 1])


def _emit_dummies(nc, reads):
    """1x1 standalone weight-loads each reading one element of a freshly-
    loaded chunk, so the PE's vector clock observes that chunk's DMA
    semaphore here (each LDW carries its single allowed wait). No PSUM
    needed; every real matmul self-loads its weights afterwards."""
    for ap in reads:
        nc.tensor.ldweights(ap)


def _emit_qk(tc, pools, head_tiles, h, g):
    """QK matmuls + exp for one group (2 query tiles, both blocks)."""
    nc = tc.nc
    (ppool, rpool, ps_pool, po_pool) = pools
    qTs, kTp, vsh, out_sb = head_tiles[h]

    # scores psum, block-major: 4 segs per block (t0-lo, t0-hi, t1-lo,
    # t1-hi). Concurrent matmuls from different PE row groups must not
    # share a PSUM bank. The middle K-chunk serves tile 2g's hi seg AND
    # tile 2g+1's lo seg, so it is one N=256 matmul: 3 matmuls + 3 weight
    # loads per block instead of 4.
    ps = ps_pool.tile([128, 2, 4, 128], F32, tag="ps")
    q0 = 256 * g
    for blk, (p0, p1) in enumerate(((0, 64), (64, 128))):
        nc.tensor.matmul(
            ps[:, blk, 0, :],
            lhsT=kTp[p0:p1, q0 : q0 + 128],
            rhs=qTs[p0:p1, q0 : q0 + 128],
            start=True,
            stop=True,
        )
        nc.tensor.matmul(
            ps[:, blk, 1:3, :],
            lhsT=kTp[p0:p1, q0 + 128 : q0 + 256],
            rhs=qTs[p0:p1, q0 : q0 + 256],
            start=True,
            stop=True,
        )
        nc.tensor.matmul(
            ps[:, blk, 3, :],
            lhsT=kTp[p0:p1, q0 + 256 : q0 + 384],
            rhs=qTs[p0:p1, q0 + 128 : q0 + 256],
            start=True,
            stop=True,
        )

    # exp((q.k)/8) for both tiles in one ScalarE pass; bf16 out. pt/pm
    # buffers are never recycled (bufs cover all groups), so exp carries
    # exactly one wait ([PE]) -- the ACTIVATE/TT/LDW ISA structs have a
    # single sync-wait slot.
    pt = ppool.tile([128, 2, 4, 128], BF16, tag="pt")
    nc.scalar.activation(
        pt[:], ps[:], mybir.ActivationFunctionType.Exp, scale=1.0 / float(D) ** 0.5
    )
    return pt


def _emit_rest(tc, pools, m_mid, head_tiles, out, h, g, pt):
    """Mask, PV, normalize, and output flush for one group."""
    nc = tc.nc
    (ppool, rpool, ps_pool, po_pool) = pools
    qTs, kTp, vsh, out_sb = head_tiles[h]

    # band mask (0/1 multiply): lo segs keep i<=jj, hi segs keep i>=jj.
    # One op over the whole [p, (b j), c, i] view; out-of-range keys on
    # the edge tiles are neutralized by the zero ones-column halo in vsh.
    # (Pool could do this in principle but measures ~62 G elem/s vs DVE's
    # ~270 -- it would become the bottleneck.)
    pm = ppool.tile([128, 2, 4, 128], BF16, tag="pm")
    nc.vector.tensor_tensor(
        pm[:].rearrange("p b (j c) i -> p (b j) c i", c=2),
        pt[:].rearrange("p b (j c) i -> p (b j) c i", c=2),
        m_mid[:, None, :, :].to_broadcast((128, 4, 2, 128)),
        mybir.AluOpType.mult,
    )

    # PV + row-sum: po[p, j, blk, :] = P_seg.T @ [V|1]
    po = po_pool.tile([128, 2, 2, 65], F32, tag="po")
    for j, t in enumerate((2 * g, 2 * g + 1)):
        for blk in range(R):
            nc.tensor.matmul(
                po[:, j, blk, :],
                lhsT=pm[:, blk, 2 * j + 0, :],
                rhs=vsh[:, t, blk, :],
                start=True,
                stop=False,
            )
            nc.tensor.matmul(
                po[:, j, blk, :],
                lhsT=pm[:, blk, 2 * j + 1, :],
                rhs=vsh[:, t + 1, blk, :],
                start=False,
                stop=True,
            )
    # normalize both tiles at once: out = po[..., 0:64] / po[..., 64]
    rc = rpool.tile([128, 2, 2], F32, tag="rc")
    nc.vector.reciprocal(rc[:], po[:, :, :, 64])
    nc.vector.tensor_tensor(
        out_sb[:, 2 * g : 2 * g + 2, :].rearrange("p t (r d) -> p t r d", r=R),
        po[:, :, :, 0:64],
        rc[:, :, :, None].to_broadcast((128, 2, R, D)),
        mybir.AluOpType.mult,
    )
    # flush finished tiles: 4-tile chunks (1KB DMA elements) at odd g,
    # except the end, where g6/g7 flush 2 tiles each so the final flush
    # after the last norm is as small as possible.
    if g % 2 == 1 and g < NT // 2 - 1:
        t0 = 2 * (g - 1)
        nc.sync.dma_start(out[h, :, t0 : t0 + 4, :], out_sb[:, t0 : t0 + 4, :])
    elif g >= NT // 2 - 2:
        t0 = 2 * g
        nc.sync.dma_start(out[h, :, t0 : t0 + 2, :], out_sb[:, t0 : t0 + 2, :])


def _build_mask(tc, mpool):
    """One [128, 2(lo|hi), 128] bf16 0/1 band-mask tile.

    Element [jj, c, i]: lo (c=0) keeps i <= jj, hi (c=1) keeps i >= jj.
    Edge tiles need no variants: out-of-range keys hit the zero halo of
    vsh's ones column, so they contribute nothing either way.
    """
    nc = tc.nc
    ge = mybir.AluOpType.is_ge
    m = mpool.tile([128, 2, 128], BF16, tag="m_mid")
    nc.gpsimd.memset(m[:], 1.0)
    # lo: keep jj - i >= 0
    nc.gpsimd.affine_select(
        m[:, 0, :], m[:, 0, :], [[-1, 128]], ge, 0.0, base=0, channel_multiplier=1
    )
    # hi: keep i - jj >= 0
    nc.gpsimd.affine_select(
        m[:, 1, :], m[:, 1, :], [[1, 128]], ge, 0.0, base=0, channel_multiplier=-1
    )
    # DVE-proc absorber: the TensorTensor ISA struct takes a single sync
    # wait, so the first real mask multiply must not combine its exp wait
    # with the one-time Pool mask-init wait. This dummy read makes the DVE
    # clock observe the final (= maximal-tick) Pool init op here.
    mdmy = mpool.tile([1, 2], BF16, tag="mdmy")
    nc.vector.tensor_tensor(
        mdmy[0:1, 0:1], m[0:1, 0, 0:1], m[0:1, 1, 0:1], mybir.AluOpType.mult
    )
    return m


def build_bass():
    nc = bacc.Bacc("TRN2", target_bir_lowering=False, debug=False)
    qT = nc.dram_tensor("qT", [HPC, R, D, NSEQ], BF16, kind="ExternalInput")
    kT = nc.dram_tensor("kT", [HPC, R, D, NSEQ + 128], BF16, kind="ExternalInput")
    v = nc.dram_tensor("v", [HPC, 128, NT + 1, R, 65], BF16, kind="ExternalInput")
    out = nc.dram_tensor("out", [HPC, 128, NT, 128], BF16, kind="ExternalOutput")

    with tile.TileContext(nc) as tc:
        with (
            tc.tile_pool(name="mpool", bufs=1) as mpool,
            tc.tile_pool(name="trans", bufs=2) as trans,
            tc.tile_pool(name="vpool", bufs=2) as vpool,
            tc.tile_pool(name="ppool", bufs=NT) as ppool,
            tc.tile_pool(name="opool", bufs=2) as opool,
            tc.tile_pool(name="rpool", bufs=8) as rpool,
            tc.tile_pool(name="ps_pool", bufs=3, space="PSUM") as ps_pool,
            tc.tile_pool(name="po_pool", bufs=2, space="PSUM") as po_pool,
        ):
            # all input DMAs ride the single SWDGE (gpsimd) queue, in
            # exact consumption order -- the 16 DMA hardware engines are
            # shared across queues, so splitting streams over queues only
            # interleaves (and delays) the critical early bytes. SWDGE's
            # ring is ~8 deep vs HWDGE's 2, so the stream rarely blocks
            # on ring slots. The first-chunk loads go out before the mask
            # build (mask isn't needed until the first exp lands, ~4us
            # after the first QK).
            head_tiles = [_alloc_tiles(trans, vpool, opool) for _ in range(HPC)]
            for h in range(HPC):
                _emit_early_loads(nc, head_tiles, qT[:], kT[:], v[:], h)
            m_mid = _build_mask(tc, mpool)
            _emit_gp_stages(nc, head_tiles, qT[:], kT[:], v[:])

            # dummy-absorber staging mirrors the chunk stages: each group's
            # matmuls only gate on the chunks it actually consumes.
            pools = (ppool, rpool, ps_pool, po_pool)

            def grp(h, g, qk_reads=None, rest_reads=None):
                if qk_reads:
                    _emit_dummies(nc, qk_reads)
                pt = _emit_qk(tc, pools, head_tiles, h, g)
                if rest_reads:
                    _emit_dummies(nc, rest_reads)
                _emit_rest(tc, pools, m_mid, head_tiles, out[:], h, g, pt)

            for h in range(HPC):
                qTs, kTp, vsh, _ = head_tiles[h]
                grp(
                    h,
                    0,
                    qk_reads=[qTs[0:1, 0:1], kTp[0:1, 0:1]],
                    rest_reads=[vsh[0:1, 0, 0, 0:1]],
                )
            for h in range(HPC):
                qTs, kTp, vsh, _ = head_tiles[h]
                grp(h, 1, qk_reads=[qTs[0:1, 256:257], kTp[0:1, QK1 : QK1 + 1]])
            for h in range(HPC):
                qTs, kTp, vsh, _ = head_tiles[h]
                grp(
                    h,
                    2,
                    qk_reads=[qTs[0:1, QK2 : QK2 + 1], kTp[0:1, QK2 : QK2 + 1]],
                    rest_reads=[vsh[0:1, V1, 0, 0:1]],
                )
            for h in range(HPC):
                grp(h, 3)
            for h in range(HPC):
                qTs, kTp, vsh, _ = head_tiles[h]
                grp(
                    h,
                    4,
                    qk_reads=[
                        qTs[0:1, QK3 : QK3 + 1],
                        kTp[0:1, QK3 : QK3 + 1],
                        vsh[0:1, V2, 0, 0:1],
                    ],
                )
            for g in range(5, NT // 2):
                for h in range(HPC):
                    grp(h, g)
    nc.compile()
    return nc


_NC_CACHE = None


def kernel(q: np.ndarray, k: np.ndarray, v: np.ndarray) -> np.ndarray:
    global _NC_CACHE
    if _NC_CACHE is None:
        _NC_CACHE = build_bass()
    nc = _NC_CACHE

    bf16 = ml_dtypes.bfloat16
    q = np.asarray(q, dtype=np.float32).astype(bf16)
    k = np.asarray(k, dtype=np.float32).astype(bf16)
    v = np.asarray(v, dtype=np.float32).astype(bf16)

    # host-side shard + relayout (all bf16 on the wire): q/k to per-offset
    # d-major [h, r, d, i], k zero-padded by 64 cols each end; v to the
    # shifted window layout [h, p, t, r, 65] with a ones column (1.0 on
    # real rows only -- the halo stays 0 so out-of-range keys vanish from
    # the softmax denominator).
    qT = q[0].reshape(H, NSEQ, R, D).transpose(0, 2, 3, 1)
    kT = k[0].reshape(H, NSEQ, R, D).transpose(0, 2, 3, 1)
    kT = np.pad(kT, ((0, 0), (0, 0), (0, 0), (HALF, HALF)))

    vpad = np.zeros((H, NSEQ + 128, R, D + 1), dtype=bf16)
    vpad[:, HALF : HALF + NSEQ, :, :D] = v[0].reshape(H, NSEQ, R, D)
    vpad[:, HALF : HALF + NSEQ, :, D] = 1.0
    # vsh[h, p, t, r, :] = vpad[h, 128t + p, r, :]
    vsh = np.empty((H, 128, NT + 1, R, D + 1), dtype=bf16)
    for t in range(NT + 1):
        vsh[:, :, t] = vpad[:, 128 * t : 128 * t + 128]

    in_maps = []
    for c in range(N_CORES):
        hs = slice(c * HPC, (c + 1) * HPC)
        in_maps.append(
            {
                "qT": np.ascontiguousarray(qT[hs]),
                "kT": np.ascontiguousarray(kT[hs]),
                "v": np.ascontiguousarray(vsh[hs]),
            }
        )

    res = run_bass_kernel_spmd(nc, in_maps, core_ids=list(range(N_CORES)))
    global LAST_RESULTS
    LAST_RESULTS = res
    out = np.empty((B, H, S, D), dtype=np.float32)
    for c in range(N_CORES):
        # device layout [h, p, t, (r d)] -> natural [(t p r), d]
        o = res.results[c]["out"].astype(np.float32)
        o = o.reshape(HPC, 128, NT, R, D).transpose(0, 2, 1, 3, 4).reshape(HPC, S, D)
        out[0, c * HPC : (c + 1) * HPC] = o
    return out


# revision 48
# speedup vs baseline: 1.0797x; 1.0797x over previous
"""Dilated attention (banded local-window attention) for Trainium2.

Problem: q,k,v [1, 16, 4096, 64] fp32; dilation r=2, window 128 (band |i-j|<=64
within each of the 2 strided subsequences of length 2048 per head).

Sharding: 16 heads x 2 offsets = 32 independent blocks -> 8 cores x 2 heads
(4 blocks). The host-side shard step hands each core its q/k already in
d-major ("transposed") per-offset layout [head, r, d, i] so the device reads
Q^T/K^T with full-bandwidth contiguous descriptors; offset r=0 (block A)
lands on SBUF partitions 0:64 and r=1 (block B) on 64:128, which feeds
row-packed K=64 QK matmuls on the two halves of the PE array. All I/O is
bf16 on the wire (the kernel computes in bf16 anyway).

Per block, queries are tiled in 16 tiles of 128; each tile attends to a
256-key window (two 128-chunks at +-64 around the tile). Scores are computed
transposed (S^T[jj, i]) so the probabilities come out pre-transposed for the
PV matmul (no on-chip transpose of P). Softmax skips the max-subtraction
(scores ~ N(0,1) after the 1/8 scale, exp is safe) and folds the 1/8 scale
into the ScalarE exp. The band mask is a 0/1 bf16 multiply after exp. Row
sums come from a ones-column appended to V; out = (P@[V|1])[:,:64] *
1/(P@[V|1])[:,64]. The ones column is zero on the +-64-row halo around the
sequence, so out-of-range keys (zero-padded in kT, exp(0)=1 in P) contribute
nothing to either numerator or denominator -- one uniform band mask suffices
for every tile, no edge-tile variants.

Schedule: the 16 DMA hardware engines are shared by every DGE queue, so
input bytes form effectively ONE ~300GB/s stream; the only scheduling that
matters is (a) dispatch early, (b) order chunks by consumption. Inputs ride
the single SWDGE (gpsimd) ring (depth ~8 vs HWDGE's 2; each dispatch costs
~0.7us) as per-head chunks in consumption order -- q/k [0:384]/[384:1152],
v tiles [0:5]/[5:9] -- except the last-needed q/k chunks [1152:], which go
on the sync HWDGE queue emitted after g1 to relieve SWDGE dispatch pacing.
The two heads' group loops are interleaved so the PE always has the other
head's QK work while one head's exp->mask->PV chain drains. Scores PSUM is
triple-buffered and PV PSUM double-buffered (3*2 + 2*1 = 8 banks exactly).
The output is written in the SBUF-native [p, t, (r d)] layout (1KB
contiguous per partition per flush; the natural [S, D] layout would be a
128B-element scatter at ~1/8th DMA efficiency) and un-permuted on the
host; the final two groups flush 2 tiles each so almost nothing trails the
last matmul.

Dummy 1x1 matmuls ("absorbers") read one element of each freshly-loaded
chunk so the PE sequencer observes the DMA semaphores there; the real
matmuls then never combine a DMA wait with their steady-state PSUM-WAW
wait (MM ISA wait-slot limit is 2). Same trick as the DVE mdmy read for
the one-time Pool mask-init wait (TT limit is 1).
"""

import sys

for _p in ("/opt/trn_rl_repo", "/opt/trn_rl_repo/concourse"):
    if _p not in sys.path:
        sys.path.insert(0, _p)

import ml_dtypes
import numpy as np

import concourse.bass as bass
import concourse.mybir as mybir
import concourse.tile as tile
from concourse import bacc
from concourse.bass_utils import run_bass_kernel_spmd


def _ensure_axon_hooks():
    """run_bass_kernel_spmd's trace path hard-imports antenv.axon_hooks,
    which some agent images lack; with BASS_TRACE set that import crashes
    kernel(). Provide a compatible stand-in whose hook is None, which the
    trace path treats as 'profiling unavailable': tracing is skipped and
    the kernel still runs. Environments with the real module (or a shim
    installed by the caller beforehand) are untouched."""
    try:
        import antenv.axon_hooks  # noqa: F401
        return
    except ImportError:
        pass
    import types

    try:
        import antenv  # noqa: F401
    except ImportError:
        return
    hooks = types.ModuleType("antenv.axon_hooks")
    hooks._HOOK = None
    hooks.set_axon_ntff_profile_hook = lambda h: setattr(hooks, "_HOOK", h)
    hooks.get_axon_ntff_profile_hook = lambda: hooks._HOOK
    sys.modules["antenv.axon_hooks"] = hooks


_ensure_axon_hooks()

N_CORES = 8
B, H, S, D = 1, 16, 4096, 64
R = 2                      # dilation rate
NSEQ = S // R              # 2048 per-offset sequence length
HALF = 64                  # window//2
NT = NSEQ // 128           # 16 query tiles per block
HPC = H // N_CORES         # heads per core = 2

F32 = mybir.dt.float32
BF16 = mybir.dt.bfloat16

# q/k column chunks and v tile chunks, in consumption order: stage 0/1
# covers group 0 (q cols <256, k cols <384, v tiles <5), stage 2/3 groups
# 1..3 (q/k cols <1152, v tiles <9), stage 4 the rest. Every engine exits
# the framework preamble at the same time (~7.8us), so there is no early-
# dispatch advantage to the HWDGE queues -- the SWDGE stream carries
# everything except the last-needed q/k chunks.
QK1, QK2 = 384, 1152
V1, V2 = 5, 9


def _alloc_tiles(trans, vpool, opool):
    qTs = trans.tile([128, NSEQ], BF16, tag="qTs")
    kTp = trans.tile([128, NSEQ + 128], BF16, tag="kTp")
    vsh = vpool.tile([128, NT + 1, R, 65], BF16, tag="vsh")
    out_sb = opool.tile([128, NT, 128], BF16, tag="out_sb")
    return qTs, kTp, vsh, out_sb


def _emit_chunk_loads(nc, head_tiles, qT, kT, v, h, stage):
    """Dispatch one consumption-stage of input DMAs for head h."""
    qTs, kTp, vsh, _ = head_tiles[h]
    qTr = qT[h].rearrange("r d i -> (r d) i")
    kTr = kT[h].rearrange("r d i -> (r d) i")
    if stage == 0:
        nc.gpsimd.dma_start(qTs[:, 0:256], qTr[:, 0:256])
        nc.gpsimd.dma_start(kTp[:, 0:QK1], kTr[:, 0:QK1])
    elif stage == 1:
        nc.gpsimd.dma_start(vsh[:, 0:V1], v[h, :, 0:V1])
    elif stage == 2:
        nc.gpsimd.dma_start(qTs[:, 256:QK2], qTr[:, 256:QK2])
        nc.gpsimd.dma_start(kTp[:, QK1:QK2], kTr[:, QK1:QK2])
    elif stage == 3:
        nc.gpsimd.dma_start(vsh[:, V1:V2], v[h, :, V1:V2])
    else:
        # stage-4 q/k ride the sync HWDGE queue, emitted after g1 so the
        # SWDGE dispatch pacing is relieved; they are only needed from g4.
        nc.sync.dma_start(qTs[:, QK2:NSEQ], qTr[:, QK2:NSEQ])
        nc.sync.dma_start(kTp[:, QK2 : NSEQ + 128], kTr[:, QK2 : NSEQ + 128])
        nc.gpsimd.dma_start(vsh[:, V2 : NT + 1], v[h, :, V2 : NT + 1])


def _emit_dummies(nc, reads):
    """1x1 standalone weight-loads each reading one element of a freshly-
    loaded chunk, so the PE's vector clock observes that chunk's DMA
    semaphore here (each LDW carries its single allowed wait). No PSUM
    needed; every real matmul self-loads its weights afterwards."""
    for ap in reads:
        nc.tensor.ldweights(ap)


def _emit_qk(tc, pools, head_tiles, h, g):
    """QK matmuls + exp for one group (2 query tiles, both blocks)."""
    nc = tc.nc
    (ppool, rpool, ps_pool, po_pool) = pools
    qTs, kTp, vsh, out_sb = head_tiles[h]

    # scores psum, block-major: 4 segs per block (t0-lo, t0-hi, t1-lo,
    # t1-hi). Concurrent matmuls from different PE row groups must not
    # share a PSUM bank. The middle K-chunk serves tile 2g's hi seg AND
    # tile 2g+1's lo seg, so it is one N=256 matmul: 3 matmuls + 3 weight
    # loads per block instead of 4.
    ps = ps_pool.tile([128, 2, 4, 128], F32, tag="ps")
    q0 = 256 * g
    for blk, (p0, p1) in enumerate(((0, 64), (64, 128))):
        nc.tensor.matmul(
            ps[:, blk, 0, :],
            lhsT=kTp[p0:p1, q0 : q0 + 128],
            rhs=qTs[p0:p1, q0 : q0 + 128],
            start=True,
            stop=True,
        )
        nc.tensor.matmul(
            ps[:, blk, 1:3, :],
            lhsT=kTp[p0:p1, q0 + 128 : q0 + 256],
            rhs=qTs[p0:p1, q0 : q0 + 256],
            start=True,
            stop=True,
        )
        nc.tensor.matmul(
            ps[:, blk, 3, :],
            lhsT=kTp[p0:p1, q0 + 256 : q0 + 384],
            rhs=qTs[p0:p1, q0 + 128 : q0 + 256],
            start=True,
            stop=True,
        )

    # exp((q.k)/8) for both tiles in one ScalarE pass; bf16 out. pt/pm
    # buffers are never recycled (bufs cover all groups), so exp carries
    # exactly one wait ([PE]) -- the ACTIVATE/TT/LDW ISA structs have a
    # single sync-wait slot.
    pt = ppool.tile([128, 2, 4, 128], BF16, tag="pt")
    nc.scalar.activation(
        pt[:], ps[:], mybir.ActivationFunctionType.Exp, scale=1.0 / float(D) ** 0.5
    )
    return pt


def _emit_rest(tc, pools, m_mid, head_tiles, out, h, g, pt):
    """Mask, PV, normalize, and output flush for one group."""
    nc = tc.nc
    (ppool, rpool, ps_pool, po_pool) = pools
    qTs, kTp, vsh, out_sb = head_tiles[h]

    # band mask (0/1 multiply): lo segs keep i<=jj, hi segs keep i>=jj.
    # One op over the whole [p, (b j), c, i] view; out-of-range keys on
    # the edge tiles are neutralized by the zero ones-column halo in vsh.
    # (Pool could do this in principle but measures ~62 G elem/s vs DVE's
    # ~270 -- it would become the bottleneck.)
    pm = ppool.tile([128, 2, 4, 128], BF16, tag="pm")
    nc.vector.tensor_tensor(
        pm[:].rearrange("p b (j c) i -> p (b j) c i", c=2),
        pt[:].rearrange("p b (j c) i -> p (b j) c i", c=2),
        m_mid[:, None, :, :].to_broadcast((128, 4, 2, 128)),
        mybir.AluOpType.mult,
    )

    # PV + row-sum: po[p, j, blk, :] = P_seg.T @ [V|1]
    po = po_pool.tile([128, 2, 2, 65], F32, tag="po")
    for j, t in enumerate((2 * g, 2 * g + 1)):
        for blk in range(R):
            nc.tensor.matmul(
                po[:, j, blk, :],
                lhsT=pm[:, blk, 2 * j + 0, :],
                rhs=vsh[:, t, blk, :],
                start=True,
                stop=False,
            )
            nc.tensor.matmul(
                po[:, j, blk, :],
                lhsT=pm[:, blk, 2 * j + 1, :],
                rhs=vsh[:, t + 1, blk, :],
                start=False,
                stop=True,
            )
    # normalize both tiles at once: out = po[..., 0:64] / po[..., 64]
    rc = rpool.tile([128, 2, 2], F32, tag="rc")
    nc.vector.reciprocal(rc[:], po[:, :, :, 64])
    nc.vector.tensor_tensor(
        out_sb[:, 2 * g : 2 * g + 2, :].rearrange("p t (r d) -> p t r d", r=R),
        po[:, :, :, 0:64],
        rc[:, :, :, None].to_broadcast((128, 2, R, D)),
        mybir.AluOpType.mult,
    )
    # flush finished tiles: 4-tile chunks (1KB DMA elements) at odd g,
    # except the end, where g6/g7 flush 2 tiles each so the final flush
    # after the last norm is as small as possible.
    if g % 2 == 1 and g < NT // 2 - 1:
        t0 = 2 * (g - 1)
        nc.sync.dma_start(out[h, :, t0 : t0 + 4, :], out_sb[:, t0 : t0 + 4, :])
    elif g >= NT // 2 - 2:
        t0 = 2 * g
        nc.sync.dma_start(out[h, :, t0 : t0 + 2, :], out_sb[:, t0 : t0 + 2, :])


def _build_mask(tc, mpool):
    """One [128, 2(lo|hi), 128] bf16 0/1 band-mask tile.

    Element [jj, c, i]: lo (c=0) keeps i <= jj, hi (c=1) keeps i >= jj.
    Edge tiles need no variants: out-of-range keys hit the zero halo of
    vsh's ones column, so they contribute nothing either way.
    """
    nc = tc.nc
    ge = mybir.AluOpType.is_ge
    m = mpool.tile([128, 2, 128], BF16, tag="m_mid")
    nc.gpsimd.memset(m[:], 1.0)
    # lo: keep jj - i >= 0
    nc.gpsimd.affine_select(
        m[:, 0, :], m[:, 0, :], [[-1, 128]], ge, 0.0, base=0, channel_multiplier=1
    )
    # hi: keep i - jj >= 0
    nc.gpsimd.affine_select(
        m[:, 1, :], m[:, 1, :], [[1, 128]], ge, 0.0, base=0, channel_multiplier=-1
    )
    # DVE-proc absorber: the TensorTensor ISA struct takes a single sync
    # wait, so the first real mask multiply must not combine its exp wait
    # with the one-time Pool mask-init wait. This dummy read makes the DVE
    # clock observe the final (= maximal-tick) Pool init op here.
    mdmy = mpool.tile([1, 2], BF16, tag="mdmy")
    nc.vector.tensor_tensor(
        mdmy[0:1, 0:1], m[0:1, 0, 0:1], m[0:1, 1, 0:1], mybir.AluOpType.mult
    )
    return m


def build_bass():
    nc = bacc.Bacc("TRN2", target_bir_lowering=False, debug=False)
    qT = nc.dram_tensor("qT", [HPC, R, D, NSEQ], BF16, kind="ExternalInput")
    kT = nc.dram_tensor("kT", [HPC, R, D, NSEQ + 128], BF16, kind="ExternalInput")
    v = nc.dram_tensor("v", [HPC, 128, NT + 1, R, 65], BF16, kind="ExternalInput")
    out = nc.dram_tensor("out", [HPC, 128, NT, 128], BF16, kind="ExternalOutput")

    with tile.TileContext(nc) as tc:
        with (
            tc.tile_pool(name="mpool", bufs=1) as mpool,
            tc.tile_pool(name="trans", bufs=2) as trans,
            tc.tile_pool(name="vpool", bufs=2) as vpool,
            tc.tile_pool(name="ppool", bufs=NT) as ppool,
            tc.tile_pool(name="opool", bufs=2) as opool,
            tc.tile_pool(name="rpool", bufs=8) as rpool,
            tc.tile_pool(name="ps_pool", bufs=3, space="PSUM") as ps_pool,
            tc.tile_pool(name="po_pool", bufs=2, space="PSUM") as po_pool,
        ):
            # all input DMAs ride the single SWDGE (gpsimd) queue, in
            # exact consumption order -- the 16 DMA hardware engines are
            # shared across queues, so splitting streams over queues only
            # interleaves (and delays) the critical early bytes. SWDGE's
            # ring is ~8 deep vs HWDGE's 2, so the stream rarely blocks
            # on ring slots. The first-chunk loads go out before the mask
            # build (mask isn't needed until the first exp lands, ~4us
            # after the first QK).
            head_tiles = [_alloc_tiles(trans, vpool, opool) for _ in range(HPC)]
            for st in (0, 1):
                for h in range(HPC):
                    _emit_chunk_loads(nc, head_tiles, qT[:], kT[:], v[:], h, st)
            m_mid = _build_mask(tc, mpool)
            for st in (2, 3):
                for h in range(HPC):
                    _emit_chunk_loads(nc, head_tiles, qT[:], kT[:], v[:], h, st)

            # dummy-absorber staging mirrors the chunk stages: each group's
            # matmuls only gate on the chunks it actually consumes.
            pools = (ppool, rpool, ps_pool, po_pool)

            def grp(h, g, qk_reads=None, rest_reads=None):
                if qk_reads:
                    _emit_dummies(nc, qk_reads)
                pt = _emit_qk(tc, pools, head_tiles, h, g)
                if rest_reads:
                    _emit_dummies(nc, rest_reads)
                _emit_rest(tc, pools, m_mid, head_tiles, out[:], h, g, pt)

            for h in range(HPC):
                qTs, kTp, vsh, _ = head_tiles[h]
                grp(
                    h,
                    0,
                    qk_reads=[qTs[0:1, 0:1], kTp[0:1, 0:1]],
                    rest_reads=[vsh[0:1, 0, 0, 0:1]],
                )
            for h in range(HPC):
                qTs, kTp, vsh, _ = head_tiles[h]
                grp(h, 1, qk_reads=[qTs[0:1, 256:257], kTp[0:1, QK1 : QK1 + 1]])
            for h in range(HPC):
                _emit_chunk_loads(nc, head_tiles, qT[:], kT[:], v[:], h, 4)
            for h in range(HPC):
                qTs, kTp, vsh, _ = head_tiles[h]
                grp(h, 2, rest_reads=[vsh[0:1, V1, 0, 0:1]])
            for h in range(HPC):
                grp(h, 3)
            for h in range(HPC):
                qTs, kTp, vsh, _ = head_tiles[h]
                grp(
                    h,
                    4,
                    qk_reads=[
                        qTs[0:1, QK2 : QK2 + 1],
                        kTp[0:1, QK2 : QK2 + 1],
                        vsh[0:1, V2, 0, 0:1],
                    ],
                )
            for g in range(5, NT // 2):
                for h in range(HPC):
                    grp(h, g)
    nc.compile()
    return nc


_NC_CACHE = None


def kernel(q: np.ndarray, k: np.ndarray, v: np.ndarray) -> np.ndarray:
    global _NC_CACHE
    if _NC_CACHE is None:
        _NC_CACHE = build_bass()
    nc = _NC_CACHE

    bf16 = ml_dtypes.bfloat16
    q = np.asarray(q, dtype=np.float32).astype(bf16)
    k = np.asarray(k, dtype=np.float32).astype(bf16)
    v = np.asarray(v, dtype=np.float32).astype(bf16)

    # host-side shard + relayout (all bf16 on the wire): q/k to per-offset
    # d-major [h, r, d, i], k zero-padded by 64 cols each end; v to the
    # shifted window layout [h, p, t, r, 65] with a ones column (1.0 on
    # real rows only -- the halo stays 0 so out-of-range keys vanish from
    # the softmax denominator).
    qT = q[0].reshape(H, NSEQ, R, D).transpose(0, 2, 3, 1)
    kT = k[0].reshape(H, NSEQ, R, D).transpose(0, 2, 3, 1)
    kT = np.pad(kT, ((0, 0), (0, 0), (0, 0), (HALF, HALF)))

    vpad = np.zeros((H, NSEQ + 128, R, D + 1), dtype=bf16)
    vpad[:, HALF : HALF + NSEQ, :, :D] = v[0].reshape(H, NSEQ, R, D)
    vpad[:, HALF : HALF + NSEQ, :, D] = 1.0
    # vsh[h, p, t, r, :] = vpad[h, 128t + p, r, :]
    vsh = np.empty((H, 128, NT + 1, R, D + 1), dtype=bf16)
    for t in range(NT + 1):
        vsh[:, :, t] = vpad[:, 128 * t : 128 * t + 128]

    in_maps = []
    for c in range(N_CORES):
        hs = slice(c * HPC, (c + 1) * HPC)
        in_maps.append(
            {
                "qT": np.ascontiguousarray(qT[hs]),
                "kT": np.ascontiguousarray(kT[hs]),
                "v": np.ascontiguousarray(vsh[hs]),
            }
        )

    res = run_bass_kernel_spmd(nc, in_maps, core_ids=list(range(N_CORES)))
    global LAST_RESULTS
    LAST_RESULTS = res
    out = np.empty((B, H, S, D), dtype=np.float32)
    for c in range(N_CORES):
        # device layout [h, p, t, (r d)] -> natural [(t p r), d]
        o = res.results[c]["out"].astype(np.float32)
        o = o.reshape(HPC, 128, NT, R, D).transpose(0, 2, 1, 3, 4).reshape(HPC, S, D)
        out[0, c * HPC : (c + 1) * HPC] = o
    return out
